# revision 2
# baseline (speedup 1.0000x reference)
"""GPT-Neo (6-layer, hidden 1024, seq 2048) forward pass on 8 TRN2 NeuronCores.

V3: head-parallel attention + sequence-parallel MLP.

Each core owns 2 heads (a 128-wide slice of the QKV projections, selected by
per-core weight data — the instruction stream is identical on all cores) and
computes attention over the FULL 2048-token sequence for those heads. This
makes causal/local-window block skipping uniform across cores: global layers
compute only the lower-triangle key blocks (72 of 128 block pairs), local
layers only the in-window band (30 pairs). Masks reduce to four tiny
shift-invariant [128, 256] tiles applied only on diagonal/band-edge blocks,
accumulated into the score PSUM by an identity matmul on the PE (no vector-
engine mask adds).

Resharding per layer: AllGather of h^T (tokens -> full sequence, f16) before
QKV, and an AllToAll of ctx^T (head-shard -> token-shard) before the output
projection. MLP and residual stream stay sequence-parallel (256 tokens/core).

Logits: vocab-sharded tied-lm-head GEMM, vocab-row-outer with pre-tiled lm
rows, two passes over tokens, f16 output (host converts to f32).
"""
import sys
import numpy as np

sys.path.insert(0, "/opt/trn_rl_repo")

import concourse.bass as bass
import concourse.tile as tile
from concourse import mybir, bacc
from concourse.bass_utils import run_bass_kernel_spmd
from concourse.masks import make_identity

NCORES = 8
T = 2048
TL = T // NCORES   # 256 tokens per core
H = 1024
HEADS = 16
HD = 64
MLP = 4096
NL = 6
WINDOW = 256
VOCAB = 50257
VSH = 6400
EPS = 1e-5
ATTN_LOCAL = [False, True, False, True, False, True]

F16 = mybir.dt.float16
F32 = mybir.dt.float32
BF16 = mybir.dt.bfloat16

NVV = VSH // 128   # 50 vocab rows per core
NQ = T // 256      # 8 query chunks of 256
RG = [list(range(NCORES))]

# mask ids: 0=diag0 (k<=q), 1=diag1 (k+128<=q), 2=m1 (q-k<128), 3=m2 (k>q)
MASK_DIAG0, MASK_DIAG1, MASK_M1, MASK_M2 = 0, 1, 2, 3


def build(n_layers=NL, with_logits=True):
    nc = bacc.Bacc(num_devices=NCORES)

    x0_e = nc.declare_dram_parameter("x0", [TL, H], F32, isOutput=False)
    wqkv_e = nc.declare_dram_parameter("wqkv", [n_layers, 128, 3 * H], F16,
                                       isOutput=False)
    wo_e = nc.declare_dram_parameter("wo", [n_layers, 128, 8 * H], F16, isOutput=False)
    wf_e = nc.declare_dram_parameter("wf", [n_layers, 4, 128, 8192], F16, isOutput=False)
    wp_e = nc.declare_dram_parameter("wp", [n_layers, 4, 128, 8192], F16, isOutput=False)
    b32_e = nc.declare_dram_parameter("b32", [n_layers, 128, 34], F32, isOutput=False)
    b16_e = nc.declare_dram_parameter("b16", [n_layers, 1, 128 + 2 * H], F16,
                                      isOutput=False)
    mk_e = nc.declare_dram_parameter("masks", [128, 4 * 256], BF16, isOutput=False)
    if with_logits:
        lm_e = nc.declare_dram_parameter("lm", [NVV, 128, H], F16, isOutput=False)
        lbt_e = nc.declare_dram_parameter("lbt", [128, NVV], F32, isOutput=False)
        out_e = nc.declare_dram_parameter("out", [VSH, T], F16, isOutput=True)
    else:
        out_e = nc.declare_dram_parameter("out", [TL, H], F32, isOutput=True)

    from contextlib import ExitStack
    with tile.TileContext(nc) as tc:
        with ExitStack() as _stk:
            _p = lambda *a, **kw: _stk.enter_context(tc.tile_pool(*a, **kw))
            constp = _p(name="const", bufs=1)
            wbigp = _p(name="wbig", bufs=2)      # [128,8192] f16 wo/wf/wp loads
            wqkvp = _p(name="wqkv", bufs=1)      # [128,3072] f16 per-core qkv slice
            hjp = _p(name="hj", bufs=8)          # [128,2048] f16 gathered h^T blocks
            hasmp = _p(name="hasm", bufs=2)      # [128,2048] f16 h^T / x^T assembly
            kqp = _p(name="kq", bufs=2)          # [128,2048] f16 k^T/q^T
            vap = _p(name="va", bufs=2)          # [128,2048] f32 v token-major
            cap = _p(name="ca", bufs=2)          # [128,2048] f16 ctx^T
            accp = _p(name="acc", bufs=4)        # [128,512] f32 mlp partials
            xresp = _p(name="xres", bufs=4)      # [128,1024] f32 residual
            hpoolp = _p(name="hpool", bufs=2)    # [128,1024] f16 ln out
            hTp = _p(name="hT", bufs=8)          # [128,256] f16 transposed acts
            cxfp = _p(name="cxf", bufs=8)        # [128,256] f16 gathered ctx^T
            evp = _p(name="ev", bufs=2)          # [128,512] bf16 exp tiles
            gtp = _p(name="gt", bufs=17)         # [128,256] f16 mlp mid
            rbp = _p(name="rb", bufs=2)          # [128,256] f32 recip bcast
            vsbp = _p(name="vsb", bufs=1)        # [128,512] f32 evicts
            smallp = _p(name="small", bufs=3)
            b16p = _p(name="b16p", bufs=2)       # [1, 2176] f16 bias rows
            oasmp = _p(name="oasm", bufs=2)      # [128,1024] f16 logits out rows
            xtgp = _p(name="xtg", bufs=16)       # [128,512] f16 gathered xT (logits)
            lmwp = _p(name="lmw", bufs=2)        # [128,1024] f16 lm rows
            ps_sc = _p(name="ps_sc", bufs=3, space="PSUM")
            ps_ctx = _p(name="ps_ctx", bufs=2, space="PSUM")
            ps_mm = _p(name="ps_mm", bufs=2, space="PSUM")
            dramp = _p(name="dram", bufs=2, space="DRAM")
            ident = constp.tile([128, 128], F16, name="ident")
            make_identity(nc, ident[:])
            ident_bf = constp.tile([128, 128], BF16, name="identbf")
            make_identity(nc, ident_bf[:])
            ones_col = constp.tile([128, 32], F32, name="ones_col")
            nc.vector.memset(ones_col[:], 1.0)
            ones_row16 = constp.tile([1, 128], F16, name="ones_row16")
            nc.vector.memset(ones_row16[:], 1.0)
            ones_row32 = constp.tile([1, 128], F32, name="ones_row32")
            nc.vector.memset(ones_row32[:], 1.0)
            eps_t = constp.tile([128, 1], F32, name="eps_t")
            nc.vector.memset(eps_t[:], EPS)
            mk_res = constp.tile([128, 4 * 256], BF16, name="mkres")
            nc.sync.dma_start(out=mk_res[:], in_=mk_e[:])

            x_cur = []
            for tt in range(2):
                xt = xresp.tile([128, H], F32, name=f"x_init{tt}", tag="x")
                nc.sync.dma_start(out=xt[:], in_=x0_e[tt * 128:(tt + 1) * 128, :])
                x_cur.append(xt)

            def layernorm_f16(xtiles, nm):
                outs = []
                for tt in range(2):
                    stats = smallp.tile([128, 2, 6], F32, name=f"st{nm}{tt}", tag="st")
                    for s in range(2):
                        nc.vector.bn_stats(out=stats[:, s, :],
                                           in_=xtiles[tt][:, s * 512:(s + 1) * 512])
                    mv = smallp.tile([128, 2], F32, name=f"mv{nm}{tt}", tag="mv")
                    nc.vector.bn_aggr(out=mv[:], in_=stats[:])
                    rstd = smallp.tile([128, 1], F32, name=f"rs{nm}{tt}", tag="rstd")
                    nc.scalar.activation(out=rstd[:], in_=mv[:, 1:2],
                                         func=mybir.ActivationFunctionType.Sqrt,
                                         bias=eps_t[:], scale=1.0)
                    nc.vector.reciprocal(out=rstd[:], in_=rstd[:])
                    h = hpoolp.tile([128, H], F16, name=f"h{nm}{tt}", tag="h")
                    nc.vector.tensor_scalar(out=h[:], in0=xtiles[tt][:],
                                            scalar1=mv[:, 0:1], scalar2=rstd[:],
                                            op0=mybir.AluOpType.subtract,
                                            op1=mybir.AluOpType.mult)
                    outs.append(h)
                return outs

            def transpose_h(htiles, nm, dst=None):
                hT = []
                for hk in range(8):
                    if dst is None:
                        t = hTp.tile([128, TL], F16, name=f"hT{nm}{hk}", tag="hT")
                    for tt in range(2):
                        pt = ps_sc.tile([128, 128], F16, name=f"ptr{nm}{hk}{tt}", tag="sc")
                        nc.tensor.transpose(pt[:], htiles[tt][:, hk * 128:(hk + 1) * 128],
                                            ident[:])
                        if dst is None:
                            nc.vector.tensor_copy(out=t[:, tt * 128:(tt + 1) * 128],
                                                  in_=pt[:])
                        else:
                            nc.vector.tensor_copy(
                                out=dst[:, hk * TL + tt * 128: hk * TL + (tt + 1) * 128],
                                in_=pt[:])
                    if dst is None:
                        hT.append(t)
                return hT

            for l in range(n_layers):
                is_local = ATTN_LOCAL[l]

                h1 = layernorm_f16(x_cur, f"l{l}a")
                hasm = hasmp.tile([128, T], F16, name=f"hasm{l}", tag="ka")
                transpose_h(h1, f"l{l}a", dst=hasm)
                bounce_h = dramp.tile([128, T], F16, name=f"bh{l}", tag="bk")
                nc.sync.dma_start(out=bounce_h[:], in_=hasm[:])
                gath_h = dramp.tile([NCORES * 128, T], F16, name=f"gh{l}", tag="gk",
                                    addr_space="Shared")
                nc.gpsimd.collective_compute("AllGather", mybir.AluOpType.bypass,
                                             replica_groups=RG,
                                             ins=[bounce_h[:]], outs=[gath_h[:]])

                b32_sb = smallp.tile([128, 34], F32, name=f"b32{l}", tag="b32")
                nc.sync.dma_start(out=b32_sb[:], in_=b32_e[l])
                b16_sb = b16p.tile([1, 128 + 2 * H], F16, name=f"b16{l}", tag="b16")
                nc.sync.dma_start(out=b16_sb[:], in_=b16_e[l])
                wqkv = wqkvp.tile([128, 3 * H], F16, name=f"wqkv{l}", tag="wqkv")
                nc.sync.dma_start(out=wqkv[:], in_=wqkv_e[l])

                hj = []
                for j in range(8):
                    t = hjp.tile([128, T], F16, name=f"hj{l}{j}", tag="hj")
                    nc.sync.dma_start(out=t[:], in_=gath_h[j * 128:(j + 1) * 128, :])
                    hj.append(t)

                # ---- k^T then q^T for my 2 heads over the full sequence ----
                kasm = kqp.tile([128, T], F16, name=f"kasm{l}", tag="kq")
                qasm = kqp.tile([128, T], F16, name=f"qasm{l}", tag="kq")
                for (dst, w0, bcol) in ((kasm, H, 1), (qasm, 0, 0)):
                    for j in range(8):
                        pq = ps_sc.tile([128, TL], F32, name=f"p{l}{w0}{j}", tag="sc")
                        for k in range(8):
                            nc.tensor.matmul(pq[:],
                                             wqkv[:, w0 + k * 128:w0 + (k + 1) * 128],
                                             hj[j][:, k * TL:(k + 1) * TL],
                                             start=(k == 0), stop=(k == 7))
                        nc.vector.tensor_scalar_add(out=dst[:, j * TL:(j + 1) * TL],
                                                    in0=pq[:],
                                                    scalar1=b32_sb[:, bcol:bcol + 1])

                # ---- v token-major [tok, ch] for my 2 heads ----
                vasm = vap.tile([128, T], F32, name=f"vasm{l}", tag="va")
                for b in range(16):
                    j, hb = b // 2, b % 2
                    pv = ps_mm.tile([128, 128], F32, name=f"pv{l}{b}", tag="mm")
                    for k in range(8):
                        nc.tensor.matmul(
                            pv[:],
                            hj[j][:, k * TL + hb * 128: k * TL + (hb + 1) * 128],
                            wqkv[:, 2 * H + k * 128: 2 * H + (k + 1) * 128],
                            start=(k == 0), stop=False)
                    nc.tensor.matmul(pv[:], ones_row16[:, 0:128], b16_sb[:, 0:128],
                                     start=False, stop=True)
                    nc.vector.tensor_copy(out=vasm[:, b * 128:(b + 1) * 128], in_=pv[:])

                # ---- attention: query-chunk outer, key-block inner ----
                casm = cap.tile([128, T], F16, name=f"casm{l}", tag="ca")
                for qq in range(NQ):
                    qsl = slice(qq * TL, (qq + 1) * TL)
                    if is_local:
                        kbs = list(range(max(0, 2 * qq - 2), 2 * qq + 2))
                    else:
                        kbs = list(range(0, 2 * qq + 2))
                    pcs = ps_ctx.tile([128, 512], F32, name=f"pcs{l}{qq}", tag="ctx")
                    nc.vector.memset(pcs[:], 0.0)
                    for kb in kbs:
                        if kb == 2 * qq:
                            mid = MASK_DIAG0
                        elif kb == 2 * qq + 1:
                            mid = MASK_DIAG1
                        elif is_local and kb == 2 * qq - 1:
                            mid = MASK_M1
                        elif is_local and kb == 2 * qq - 2:
                            mid = MASK_M2
                        else:
                            mid = None
                        ksl = slice(kb * 128, (kb + 1) * 128)
                        ev = evp.tile([128, 2 * TL], F32, name=f"ev{l}{qq}{kb}",
                                      tag="ev")
                        if mid is None:
                            s0 = ps_sc.tile([128, TL], F32, name=f"s0{l}{qq}{kb}",
                                            tag="sc")
                            s1 = ps_sc.tile([128, TL], F32, name=f"s1{l}{qq}{kb}",
                                            tag="sc")
                            nc.tensor.matmul(s0[:], kasm[0:64, ksl],
                                             qasm[0:64, qsl], start=True, stop=True)
                            nc.tensor.matmul(s1[:], kasm[64:128, ksl],
                                             qasm[64:128, qsl], start=True, stop=True)
                            nc.scalar.activation(out=ev[:, 0:TL], in_=s0[:],
                                                 func=mybir.ActivationFunctionType.Exp)
                            nc.scalar.activation(out=ev[:, TL:2 * TL], in_=s1[:],
                                                 func=mybir.ActivationFunctionType.Exp)
                        else:
                            msl = slice(mid * 256, (mid + 1) * 256)
                            s0 = ps_sc.tile([128, TL], F32, name=f"s0{l}{qq}{kb}",
                                            tag="sc")
                            s1 = ps_sc.tile([128, TL], F32, name=f"s1{l}{qq}{kb}",
                                            tag="sc")
                            nc.tensor.matmul(s0[:], ident_bf[:], mk_res[:, msl],
                                             start=True, stop=False)
                            nc.tensor.matmul(s0[:], kasm[0:64, ksl], qasm[0:64, qsl],
                                             start=False, stop=True)
                            nc.tensor.matmul(s1[:], ident_bf[:], mk_res[:, msl],
                                             start=True, stop=False)
                            nc.tensor.matmul(s1[:], kasm[64:128, ksl],
                                             qasm[64:128, qsl],
                                             start=False, stop=True)
                            nc.scalar.activation(out=ev[:, 0:TL], in_=s0[:],
                                                 func=mybir.ActivationFunctionType.Exp)
                            nc.scalar.activation(out=ev[:, TL:2 * TL], in_=s1[:],
                                                 func=mybir.ActivationFunctionType.Exp)
                        sp = (kb == kbs[-1])
                        nc.tensor.matmul(pcs[0:64, 0:TL],
                                         vasm[:, kb * 128: kb * 128 + 64],
                                         ev[:, 0:TL],
                                         start=False, stop=sp, tile_position=(0, 0),
                                         skip_group_check=True)
                        nc.tensor.matmul(pcs[64:128, 0:TL],
                                         vasm[:, kb * 128 + 64: (kb + 1) * 128],
                                         ev[:, TL:2 * TL],
                                         start=False, stop=sp, tile_position=(0, 64),
                                         skip_group_check=True)
                        nc.tensor.matmul(pcs[0:32, TL:2 * TL], ones_col[:],
                                         ev[:, 0:TL],
                                         start=False, stop=sp, tile_position=(0, 0),
                                         skip_group_check=True)
                        nc.tensor.matmul(pcs[32:64, TL:2 * TL], ones_col[:],
                                         ev[:, TL:2 * TL],
                                         start=False, stop=sp, tile_position=(0, 32),
                                         skip_group_check=True)
                    rsA = smallp.tile([1, TL], F32, name=f"rsA{l}{qq}", tag="rsA")
                    rsB = smallp.tile([1, TL], F32, name=f"rsB{l}{qq}", tag="rsB")
                    nc.vector.reciprocal(out=rsA[:], in_=pcs[0:1, TL:2 * TL])
                    nc.vector.reciprocal(out=rsB[:], in_=pcs[32:33, TL:2 * TL])
                    pbc = ps_sc.tile([128, TL], F32, name=f"pbc{l}{qq}", tag="sc")
                    nc.tensor.matmul(pbc[0:64, :], ones_row32[:, 0:64], rsA[:],
                                     start=True, stop=True, tile_position=(0, 0))
                    nc.tensor.matmul(pbc[64:128, :], ones_row32[:, 0:64], rsB[:],
                                     start=True, stop=True, tile_position=(0, 64))
                    rb = rbp.tile([128, TL], F32, name=f"rb{l}{qq}", tag="rb")
                    nc.vector.tensor_copy(out=rb[:], in_=pbc[:])
                    nc.vector.tensor_tensor(out=casm[:, qsl], in0=pcs[:, 0:TL],
                                            in1=rb[:], op=mybir.AluOpType.mult)

                # ---- AllToAll ctx^T: head-shard -> token-shard ----
                bounce_c = dramp.tile([NCORES * 128, TL], F16, name=f"bc{l}", tag="bc")
                for j in range(8):
                    nc.sync.dma_start(out=bounce_c[j * 128:(j + 1) * 128, :],
                                      in_=casm[:, j * TL:(j + 1) * TL])
                a2a_c = dramp.tile([NCORES * 128, TL], F16, name=f"ac{l}", tag="ac")
                nc.gpsimd.collective_compute("AllToAll", mybir.AluOpType.bypass,
                                             replica_groups=RG,
                                             ins=[bounce_c[:]], outs=[a2a_c[:]])
                cxf = []
                for k in range(8):
                    t = cxfp.tile([128, TL], F16, name=f"cx{l}{k}", tag="cx")
                    nc.sync.dma_start(out=t[:], in_=a2a_c[k * 128:(k + 1) * 128, :])
                    cxf.append(t)

                # ---- attention out projection + residual ----
                wot = wbigp.tile([128, 8 * H], F16, name=f"wo{l}", tag="w")
                nc.sync.dma_start(out=wot[:], in_=wo_e[l])
                x_new = []
                for tt in range(2):
                    xt = xresp.tile([128, H], F32, name=f"xa{l}{tt}", tag="x")
                    for nn in range(2):
                        pa = ps_mm.tile([128, 512], F32, name=f"pa{l}{tt}{nn}", tag="mm")
                        for k in range(8):
                            nc.tensor.matmul(pa[:], cxf[k][:, tt * 128:(tt + 1) * 128],
                                             wot[:, k * H + nn * 512: k * H + (nn + 1) * 512],
                                             start=(k == 0), stop=False)
                        nc.tensor.matmul(pa[:], ones_row16[:, 0:128],
                                         b16_sb[:, 128 + nn * 512: 128 + (nn + 1) * 512],
                                         start=False, stop=True)
                        nc.vector.tensor_tensor(out=xt[:, nn * 512:(nn + 1) * 512],
                                                in0=pa[:],
                                                in1=x_cur[tt][:, nn * 512:(nn + 1) * 512],
                                                op=mybir.AluOpType.add)
                    x_new.append(xt)
                x_cur = x_new

                # ---- MLP (two halves of the 4096 dim) ----
                h2 = layernorm_f16(x_cur, f"l{l}b")
                h2T = transpose_h(h2, f"l{l}b")
                x_new = [xresp.tile([128, H], F32, name=f"xm{l}{tt}", tag="x")
                         for tt in range(2)]
                part_sb = [[None, None], [None, None]]
                for halfk in range(2):
                    gts = []
                    for g in range(2):
                        wfg = wbigp.tile([128, 8192], F16, name=f"wf{l}{halfk}{g}", tag="w")
                        nc.sync.dma_start(out=wfg[:], in_=wf_e[l, halfk * 2 + g])
                        for ofh in range(8):
                            of = (halfk * 2 + g) * 8 + ofh
                            pf = ps_sc.tile([128, TL], F32, name=f"pf{l}{of}", tag="sc")
                            for k in range(8):
                                nc.tensor.matmul(
                                    pf[:],
                                    wfg[:, ofh * H + k * 128: ofh * H + (k + 1) * 128],
                                    h2T[k][:], start=(k == 0), stop=(k == 7))
                            gtt = gtp.tile([128, TL], F16, name=f"g{l}{of}", tag="g")
                            nc.scalar.activation(out=gtt[:], in_=pf[:],
                                                 func=mybir.ActivationFunctionType.Gelu,
                                                 bias=b32_sb[:, 2 + of:3 + of], scale=1.0)
                            gts.append(gtt)
                    for nn in range(2):
                        wpg = wbigp.tile([128, 8192], F16, name=f"wp{l}{halfk}{nn}", tag="w")
                        nc.sync.dma_start(out=wpg[:], in_=wp_e[l, halfk * 2 + nn])
                        for tt in range(2):
                            pp = ps_mm.tile([128, 512], F32, name=f"pp{l}{halfk}{tt}{nn}",
                                            tag="mm")
                            for kk in range(16):
                                nc.tensor.matmul(pp[:], gts[kk][:, tt * 128:(tt + 1) * 128],
                                                 wpg[:, kk * 512:(kk + 1) * 512],
                                                 start=(kk == 0),
                                                 stop=(halfk == 0 and kk == 15))
                            if halfk == 0:
                                s = accp.tile([128, 512], F32, name=f"ph{l}{tt}{nn}",
                                              tag="acc")
                                nc.vector.tensor_copy(out=s[:], in_=pp[:])
                                part_sb[tt][nn] = s
                            else:
                                nc.tensor.matmul(
                                    pp[:], ones_row16[:, 0:128],
                                    b16_sb[:, 128 + H + nn * 512:
                                           128 + H + (nn + 1) * 512],
                                    start=False, stop=True)
                                t2 = vsbp.tile([128, 512], F32, name=f"pj{l}{tt}{nn}",
                                               tag="vsb")
                                nc.vector.tensor_tensor(out=t2[:], in0=pp[:],
                                                        in1=part_sb[tt][nn][:],
                                                        op=mybir.AluOpType.add)
                                nc.vector.tensor_tensor(
                                    out=x_new[tt][:, nn * 512:(nn + 1) * 512],
                                    in0=t2[:],
                                    in1=x_cur[tt][:, nn * 512:(nn + 1) * 512],
                                    op=mybir.AluOpType.add)
                x_cur = x_new

            if not with_logits:
                for tt in range(2):
                    nc.sync.dma_start(out=out_e[tt * 128:(tt + 1) * 128, :], in_=x_cur[tt][:])
            else:
                xh = layernorm_f16(x_cur, "f")
                xasm = hasmp.tile([128, T], F16, name="xasm", tag="ka")
                transpose_h(xh, "f", dst=xasm)
                bounce_x = dramp.tile([128, T], F16, name="bx", tag="bk")
                nc.sync.dma_start(out=bounce_x[:], in_=xasm[:])
                gath_x = dramp.tile([NCORES * 128, T], F16, name="gx", tag="gk",
                                    addr_space="Shared")
                nc.gpsimd.collective_compute("AllGather", mybir.AluOpType.bypass,
                                             replica_groups=RG,
                                             ins=[bounce_x[:]], outs=[gath_x[:]])
                lbt_sb = smallp.tile([128, NVV], F32, name="lbt", tag="lbt")
                nc.sync.dma_start(out=lbt_sb[:], in_=lbt_e[:])
                for hp4 in range(2):
                    xtgc = []
                    for tc in range(2):
                        tc4 = hp4 * 2 + tc
                        for k in range(8):
                            t = xtgp.tile([128, 512], F16, name=f"xtg{tc4}{k}", tag="xtg")
                            for j in range(2):
                                cc = tc4 * 2 + j
                                nc.sync.dma_start(
                                    out=t[:, j * TL:(j + 1) * TL],
                                    in_=gath_x[cc * 128:(cc + 1) * 128,
                                               k * TL:(k + 1) * TL])
                            xtgc.append(t)
                    for vv in range(NVV):
                        lmw = lmwp.tile([128, H], F16, name=f"lm{hp4}{vv}", tag="lm")
                        nc.sync.dma_start(out=lmw[:], in_=lm_e[vv])
                        o = oasmp.tile([128, 2 * 512], F16, name=f"o{hp4}{vv}", tag="o")
                        for tc in range(2):
                            pl = (ps_mm if tc == 0 else ps_ctx).tile(
                                [128, 512], F32, name=f"pl{hp4}{vv}{tc}",
                                tag=("mm" if tc == 0 else "ctx"))
                            for k in range(8):
                                nc.tensor.matmul(pl[:], lmw[:, k * 128:(k + 1) * 128],
                                                 xtgc[tc * 8 + k][:],
                                                 start=(k == 0), stop=(k == 7))
                            nc.vector.tensor_scalar_add(out=o[:, tc * 512:(tc + 1) * 512],
                                                        in0=pl[:],
                                                        scalar1=lbt_sb[:, vv:vv + 1])
                        nc.sync.dma_start(
                            out=out_e[vv * 128:(vv + 1) * 128,
                                      hp4 * 1024:(hp4 + 1) * 1024],
                            in_=o[:])

    nc.finalize()
    return nc


# ------------------- host-side prep & entry -------------------

def _prep_inputs(inputs, n_layers=NL, with_logits=True):
    f32 = np.float32
    f16 = np.float16
    import ml_dtypes
    bf16 = ml_dtypes.bfloat16

    ids = np.asarray(inputs["input_ids"]).reshape(-1).astype(np.int64)
    wte = np.asarray(inputs["wte"], f32)
    wpe = np.asarray(inputs["wpe"], f32)
    x0 = wte[ids] + wpe[:T]

    def row_major(w):
        # [1024, N] -> [128, k*N + j] with 8 k-blocks of 128 rows
        n = w.shape[1]
        return np.ascontiguousarray(
            w.reshape(8, 128, n).transpose(1, 0, 2).reshape(128, 8 * n))

    # shared (head-independent) weights
    wo = np.empty((n_layers, 128, 8 * H), f16)
    wf = np.empty((n_layers, 4, 128, 8192), f16)
    wp = np.empty((n_layers, 4, 128, 8192), f16)
    fb32 = np.zeros((n_layers, 128, 32), f32)
    ob16 = np.zeros((n_layers, 2 * H), f16)
    # per-core qkv slices and biases
    wqkv_c = np.empty((NCORES, n_layers, 128, 3 * H), f16)
    qkb_c = np.zeros((NCORES, n_layers, 128, 2), f32)
    vb_c = np.zeros((NCORES, n_layers, 128), f16)

    for l in range(n_layers):
        ln1w = np.asarray(inputs["ln1_w"][l], f32); ln1b = np.asarray(inputs["ln1_b"][l], f32)
        ln2w = np.asarray(inputs["ln2_w"][l], f32); ln2b = np.asarray(inputs["ln2_b"][l], f32)
        qw = (ln1w[:, None] * np.asarray(inputs["q_w"][l], f32))
        kw = (ln1w[:, None] * np.asarray(inputs["k_w"][l], f32))
        vw = (ln1w[:, None] * np.asarray(inputs["v_w"][l], f32))
        qb = ln1b @ np.asarray(inputs["q_w"][l], f32)
        kb = ln1b @ np.asarray(inputs["k_w"][l], f32)
        vb = ln1b @ np.asarray(inputs["v_w"][l], f32)
        for c in range(NCORES):
            sl = slice(c * 128, (c + 1) * 128)
            wqkv_c[c, l, :, 0:H] = row_major(qw[:, sl].astype(f16))
            wqkv_c[c, l, :, H:2 * H] = row_major(kw[:, sl].astype(f16))
            wqkv_c[c, l, :, 2 * H:3 * H] = row_major(vw[:, sl].astype(f16))
            qkb_c[c, l, :, 0] = qb[sl]
            qkb_c[c, l, :, 1] = kb[sl]
            vb_c[c, l] = vb[sl].astype(f16)
        wo[l] = row_major(np.asarray(inputs["o_w"][l], f32).astype(f16))
        ob16[l, 0:H] = np.asarray(inputs["o_b"][l], f32).astype(f16)
        w = np.asarray(inputs["fc_w"][l], f32)
        wff = (ln2w[:, None] * w).astype(f16)
        wf[l] = wff.reshape(8, 128, 4, 8, 128).transpose(2, 1, 3, 0, 4).reshape(
            4, 128, 8192)
        fbv = np.asarray(inputs["fc_b"][l], f32) + ln2b @ w
        fb32[l] = fbv.reshape(32, 128).T
        w = np.asarray(inputs["proj_w"][l], f32).astype(f16)
        wp[l] = w.reshape(2, 16, 128, 2, 512).transpose(0, 3, 2, 1, 4).reshape(
            4, 128, 8192)
        ob16[l, H:2 * H] = np.asarray(inputs["proj_b"][l], f32).astype(f16)

    # shift-invariant masks [128 key, 256 query]
    kk = np.arange(128)[:, None]
    qv = np.arange(256)[None, :]
    masks = np.zeros((128, 4 * 256), np.float32)
    masks[:, 0 * 256:1 * 256] = np.where(kk <= qv, 0.0, -30000.0)          # diag0
    masks[:, 1 * 256:2 * 256] = np.where(kk + 128 <= qv, 0.0, -30000.0)    # diag1
    masks[:, 2 * 256:3 * 256] = np.where(qv - kk < 128, 0.0, -30000.0)     # m1
    masks[:, 3 * 256:4 * 256] = np.where(kk > qv, 0.0, -30000.0)           # m2
    masks = masks.astype(bf16)

    lnfw = np.asarray(inputs["lnf_w"], f32); lnfb = np.asarray(inputs["lnf_b"], f32)
    VP = NCORES * VSH
    lm_pad = np.zeros((VP, H), f32)
    lm_pad[:VOCAB] = wte * lnfw[None, :]
    lb_pad = np.zeros((VP,), f32)
    lb_pad[:VOCAB] = wte @ lnfb

    in_maps = []
    for c in range(NCORES):
        ts = c * TL
        b32 = np.zeros((n_layers, 128, 34), f32)
        b32[:, :, 0:2] = qkb_c[c]
        b32[:, :, 2:34] = fb32
        b16 = np.zeros((n_layers, 1, 128 + 2 * H), f16)
        b16[:, 0, 0:128] = vb_c[c]
        b16[:, 0, 128:128 + 2 * H] = ob16
        m = {
            "x0": np.ascontiguousarray(x0[ts:ts + TL]).astype(f32),
            "wqkv": wqkv_c[c], "wo": wo, "wf": wf, "wp": wp,
            "b32": b32, "b16": b16, "masks": masks,
        }
        if with_logits:
            lmT = lm_pad[c * VSH:(c + 1) * VSH].T
            m["lm"] = np.ascontiguousarray(
                lmT.reshape(8, 128, NVV, 128).transpose(2, 1, 0, 3).reshape(
                    NVV, 128, H)).astype(f16)
            m["lbt"] = np.ascontiguousarray(
                lb_pad[c * VSH:(c + 1) * VSH].reshape(NVV, 128).T)
        in_maps.append(m)
    return in_maps


_NC_CACHE = {}


def _get_nc(n_layers=NL, with_logits=True):
    key = (n_layers, with_logits)
    if key not in _NC_CACHE:
        _NC_CACHE[key] = build(n_layers, with_logits)
    return _NC_CACHE[key]


def run(inputs, n_layers=NL, with_logits=True, trace=False):
    nc = _get_nc(n_layers, with_logits)
    in_maps = _prep_inputs(inputs, n_layers, with_logits)
    res = run_bass_kernel_spmd(nc, in_maps, list(range(NCORES)), trace=trace)
    if with_logits:
        parts = [res.results[c]["out"].astype(np.float32) for c in range(NCORES)]
        full = np.concatenate(parts, axis=0)[:VOCAB]
        out = np.ascontiguousarray(full.T)[None]
    else:
        out = np.concatenate([res.results[c]["out"] for c in range(NCORES)], axis=0)[None]
    return out, res


def kernel(**inputs) -> np.ndarray:
    out, _ = run(inputs, NL, True, trace=False)
    return out
